# revision 19
# baseline (speedup 1.0000x reference)
"""Distributed MultiHeadAttention (+residual, +LayerNorm) Trainium2 kernel.

Problem: B=2, S=2048, D_MODEL=1024, N_HEAD=16, D_K=D_V=64, eps=1e-6.
  qh = q@Wq, kh = k@Wk, vh = v@Wv  (per head)
  attn = softmax(qh·kh^T / 8)
  out = (attn@vh) @ Wfc + bfc + q  -> LayerNorm(gamma, beta)

Sharding: 8 cores; core c owns 512 q-rows of batch c//4 (sequence shard).
Each core projects K/V for its own 512 rows; an AllGather over each
4-core batch group materializes the full-batch K^T/V; attention, fc and
LayerNorm are then fully local (no further collectives).

All matmuls run in float32r (1 cyc/row on PE vs 4 for fp32); the
residual + LayerNorm path stays fp32. Softmax denominators come free
from a ones-column appended to the V stationary tiles.
"""

import sys

sys.path.insert(0, "/opt/trn_rl_repo")

import numpy as np

import concourse.bass as bass
import concourse.tile as tile
from concourse import bacc, mybir
from concourse.bass_utils import run_bass_kernel_spmd

N_CORES = 8
B = 2
S = 2048
D = 1024  # d_model
H = 16  # heads
DK = 64  # head dim
SS = S // 4  # 512 q-rows per core
LN_EPS = 1e-6
F32 = mybir.dt.float32
F32R = mybir.dt.float32r

DEBUG_NO_PBCAST = False

KH_SZ = D * SS  # flat f32 elements of khT_local [1024, 512]
VH_SZ = SS * D  # flat elements of vh_local [512, 1024]
CC_SZ = KH_SZ + VH_SZ


def build_kernel():
    nc = bacc.Bacc()

    qT = nc.dram_tensor("qT", [D, SS], F32R, kind="ExternalInput")
    kT = nc.dram_tensor("kT", [D, SS], F32R, kind="ExternalInput")
    vT = nc.dram_tensor("vT", [D, SS], F32R, kind="ExternalInput")
    qn = nc.dram_tensor("qn", [SS, D], F32, kind="ExternalInput")  # residual + bfc
    Wq = nc.dram_tensor("Wq", [D, D], F32R, kind="ExternalInput")
    Wk = nc.dram_tensor("Wk", [D, D], F32R, kind="ExternalInput")
    Wv = nc.dram_tensor("Wv", [D, D], F32R, kind="ExternalInput")
    Wfc = nc.dram_tensor("Wfc", [D, D], F32R, kind="ExternalInput")
    gb = nc.dram_tensor("gb", [128, D], F32, kind="ExternalInput")  # gamma bcast
    bb = nc.dram_tensor("bb", [128, D], F32, kind="ExternalInput")  # beta bcast
    onesc = nc.dram_tensor("onesc", [128, H], F32R, kind="ExternalInput")
    out = nc.dram_tensor("out", [SS, D], F32, kind="ExternalOutput")

    cc_in = nc.dram_tensor("cc_in", [CC_SZ], F32R)
    cc_out = nc.dram_tensor("cc_out", [4 * CC_SZ], F32R)

    def cc_in_kh(i):  # [128, 512] view of khT_local rows 128i..128i+127
        return cc_in[128 * SS * i : 128 * SS * (i + 1)].rearrange(
            "(p f) -> p f", f=SS
        )

    def cc_in_vh(s):  # [128, 1024] view of vh_local rows 128s..128s+127
        ofs = KH_SZ + 128 * D * s
        return cc_in[ofs : ofs + 128 * D].rearrange("(p f) -> p f", f=D)

    def cc_out_kh(c, i):  # batch-chunk c, d-rows 128i.. -> [128, 512]
        ofs = c * CC_SZ + 128 * SS * i
        return cc_out[ofs : ofs + 128 * SS].rearrange("(p f) -> p f", f=SS)

    def cc_out_vh(c, s):  # batch-chunk c, k-rows 128s.. -> [128, 1024]
        ofs = c * CC_SZ + KH_SZ + 128 * D * s
        return cc_out[ofs : ofs + 128 * D].rearrange("(p f) -> p f", f=D)

    with tile.TileContext(nc) as tc:
        with (
            tc.tile_pool(name="qhT", bufs=1) as qhT_pool,
            tc.tile_pool(name="outT", bufs=1) as outT_pool,
        ):
            _build_body(nc, tc, qhT_pool, outT_pool, locals())
    nc.compile()
    return nc


def _build_body(nc, tc, qhT_pool, outT_pool, env):
    qT = env["qT"]; kT = env["kT"]; vT = env["vT"]; qn = env["qn"]
    Wq = env["Wq"]; Wk = env["Wk"]; Wv = env["Wv"]; Wfc = env["Wfc"]
    gb = env["gb"]; bb = env["bb"]; out = env["out"]; onesc = env["onesc"]
    cc_in = env["cc_in"]; cc_out = env["cc_out"]
    cc_in_kh = env["cc_in_kh"]; cc_in_vh = env["cc_in_vh"]
    cc_out_kh = env["cc_out_kh"]; cc_out_vh = env["cc_out_vh"]

    if True:
        # ---------------- Phase A: QKV projections of own 512 rows -------
        with (
            tc.tile_pool(name="xin", bufs=1) as xin_pool,
            tc.tile_pool(name="wstat", bufs=6) as wstat_pool,
            tc.tile_pool(name="wmov", bufs=3) as wmov_pool,
            tc.tile_pool(name="stage", bufs=4) as stage_pool,
            tc.tile_pool(name="pp", bufs=2, space="PSUM") as pp_pool,
        ):
            # resident input tiles [128, 512] per in-chunk
            qT_t, kT_t, vT_t = [], [], []
            for i in range(8):
                for lst, src, tag in ((qT_t, qT, "q"), (kT_t, kT, "k"), (vT_t, vT, "v")):
                    t = xin_pool.tile([128, SS], F32R, tag=f"x{tag}{i}")
                    nc.sync.dma_start(t[:], src[128 * i : 128 * (i + 1), :])
                    lst.append(t)

            # K projection: khT_local[d, k] = Wk[in, d].T @ kT[in, k]
            # then Q the same; stationary = W tile [128 in, 128 d]
            qhT_tiles = []
            for proj, (W, xt) in enumerate(((Wk, kT_t), (Wq, qT_t))):
                for dchunk in range(8):
                    ps = pp_pool.tile([128, SS], F32, tag="pproj")
                    for i in range(8):
                        wt = wstat_pool.tile([128, 128], F32R, tag="wstat")
                        nc.sync.dma_start(
                            wt[:],
                            W[128 * i : 128 * (i + 1), 128 * dchunk : 128 * (dchunk + 1)],
                        )
                        nc.tensor.matmul(
                            ps[:], wt[:], xt[i][:], start=(i == 0), stop=(i == 7)
                        )
                    if proj == 0:  # K -> stage to DRAM for AllGather
                        st = stage_pool.tile([128, SS], F32R, tag="stagek")
                        nc.vector.tensor_copy(st[:], ps[:])
                        nc.sync.dma_start(cc_in_kh(dchunk), st[:])
                    else:  # Q -> resident SBUF
                        qt = qhT_pool.tile([128, SS], F32R, tag=f"qh{dchunk}")
                        nc.scalar.copy(qt[:], ps[:])
                        qhT_tiles.append(qt)

            # V projection: vh_local[k, d] = vT[in, k].T @ Wv[in, d]
            # stationary = vT tile [128 in, 128 k], moving = Wv rows
            for s in range(4):  # k-subtile of own 512 rows
                ps = pp_pool.tile([128, D], F32, tag="pv")
                for i in range(8):
                    wm = wmov_pool.tile([128, D], F32R, tag="wvmov")
                    nc.sync.dma_start(wm[:], Wv[128 * i : 128 * (i + 1), :])
                    stat = vT_t[i][:, 128 * s : 128 * (s + 1)]
                    nc.tensor.matmul(
                        ps[:, 0:512], stat, wm[:, 0:512], start=(i == 0), stop=(i == 7)
                    )
                    nc.tensor.matmul(
                        ps[:, 512:1024], stat, wm[:, 512:1024], start=(i == 0), stop=(i == 7)
                    )
                st = stage_pool.tile([128, D], F32R, tag="stagev")
                nc.vector.tensor_copy(st[:], ps[:])
                nc.sync.dma_start(cc_in_vh(s), st[:])

        # ---------------- AllGather K^T/V across the batch group ---------
        nc.gpsimd.collective_compute(
            "AllGather",
            mybir.AluOpType.bypass,
            replica_groups=[[0, 1, 2, 3], [4, 5, 6, 7]],
            ins=[cc_in[:]],
            outs=[cc_out[:]],
        )

        # ---------------- Phase B: attention ----------------------------
        # vh_full resident in SBUF with ones column per head: [128, 16*65]
        with (
            tc.tile_pool(name="vh", bufs=1) as vh_pool,
            tc.tile_pool(name="khT", bufs=6) as khT_pool,
            tc.tile_pool(name="pT", bufs=3) as pT_pool,
            tc.tile_pool(name="attn_sc", bufs=3, space="PSUM") as sc_pool,
            tc.tile_pool(name="attn_o", bufs=2, space="PSUM") as po_pool,
            tc.tile_pool(name="dn", bufs=4) as dn_pool,
        ):
            vh_t = []
            for j in range(16):  # global k-tile j = chunk c= j//4, sub s= j%4
                t = vh_pool.tile([128, H * 65], F32R, tag=f"vh{j}")
                dst = t[:].rearrange("p (h e) -> p h e", h=H)
                nc.sync.dma_start(
                    dst[:, :, 0:64],
                    cc_out_vh(j // 4, j % 4).rearrange("p (h e) -> p h e", h=H),
                )
                nc.sync.dma_start(dst[:, :, 64], onesc[:])
                vh_t.append(t)

            outT_tiles = []
            for i in range(8):
                oT = outT_pool.tile([128, SS], F32R, tag=f"oT{i}")
                outT_tiles.append(oT)

            for h in range(H):
                # khT stationary tiles for this head-pair, streamed
                if h % 2 == 0:
                    kh_t = []
                    for c in range(4):
                        t = khT_pool.tile([128, SS], F32R, tag="khT")
                        nc.sync.dma_start(t[:], cc_out_kh(c, h // 2))
                        kh_t.append(t)
                hofs = 64 * (h % 2)
                qmov = qhT_tiles[h // 2][hofs : hofs + 64, :]

                po = po_pool.tile([65, SS], F32, tag="po")
                for g in range(8):  # score groups of 2 k-tiles
                    ps = sc_pool.tile([128, 2 * SS], F32, tag="ps")
                    for u in range(2):
                        j = 2 * g + u
                        stat = kh_t[j // 4][hofs : hofs + 64, 128 * (j % 4) : 128 * (j % 4) + 128]
                        nc.tensor.matmul(
                            ps[:, SS * u : SS * (u + 1)], stat, qmov,
                            start=True, stop=True,
                        )
                    pt = pT_pool.tile([128, 2 * SS], F32R, tag="pT")
                    nc.scalar.activation(
                        pt[:], ps[:], mybir.ActivationFunctionType.Exp, scale=0.125
                    )
                    for u in range(2):
                        j = 2 * g + u
                        stat = vh_t[j][:, 65 * h : 65 * h + 65]
                        nc.tensor.matmul(
                            po[:], stat, pt[:, SS * u : SS * (u + 1)],
                            start=(j == 0), stop=(j == 15),
                        )

                # normalize by softmax denominator (row 64 of po)
                rec = dn_pool.tile([1, SS], F32, tag="rec")
                nc.vector.reciprocal(rec[:], po[64:65, :])
                rb = dn_pool.tile([64, SS], F32, tag="rb")
                if DEBUG_NO_PBCAST:
                    nc.vector.memset(rb[:], 1.0)
                else:
                    nc.gpsimd.partition_broadcast(rb[:], rec[:])
                dst = outT_tiles[h // 2][hofs : hofs + 64, :]
                nc.vector.tensor_mul(dst, po[0:64, :], rb[:])

        # ---------------- Phase C: fc + residual + LayerNorm -------------
        with (
            tc.tile_pool(name="wfc", bufs=8) as wfc_pool,
            tc.tile_pool(name="res", bufs=2) as res_pool,
            tc.tile_pool(name="lnc", bufs=1) as lnc_pool,
            tc.tile_pool(name="lns", bufs=2) as lns_pool,
            tc.tile_pool(name="pfc", bufs=2, space="PSUM") as pfc_pool,
        ):
            gbt = lnc_pool.tile([128, D], F32, tag="gb")
            nc.sync.dma_start(gbt[:], gb[:])
            bbt = lnc_pool.tile([128, D], F32, tag="bb")
            nc.sync.dma_start(bbt[:], bb[:])
            epst = lnc_pool.tile([128, 1], F32, tag="eps")
            nc.vector.memset(epst[:], LN_EPS)

            wfc_t = []  # moving tiles [128 hd-chunk, 1024 m]
            for i in range(8):
                t = wfc_pool.tile([128, D], F32R, tag="wfc")
                nc.sync.dma_start(t[:], Wfc[128 * i : 128 * (i + 1), :])
                wfc_t.append(t)

            for qs in range(4):  # q-subtile of 128 rows
                pf = pfc_pool.tile([128, D], F32, tag="pf")
                for i in range(8):
                    stat = outT_tiles[i][:, 128 * qs : 128 * (qs + 1)]
                    nc.tensor.matmul(
                        pf[:, 0:512], stat, wfc_t[i][:, 0:512],
                        start=(i == 0), stop=(i == 7),
                    )
                    nc.tensor.matmul(
                        pf[:, 512:1024], stat, wfc_t[i][:, 512:1024],
                        start=(i == 0), stop=(i == 7),
                    )

                rt = res_pool.tile([128, D], F32, tag="res")
                nc.sync.dma_start(rt[:], qn[128 * qs : 128 * (qs + 1), :])

                x = lns_pool.tile([128, D], F32, tag="x")
                nc.vector.tensor_add(x[:], pf[:], rt[:])
                msum = lns_pool.tile([128, 1], F32, tag="msum")
                nc.vector.reduce_sum(out=msum[:], in_=x[:], axis=mybir.AxisListType.X)
                nmu = lns_pool.tile([128, 1], F32, tag="nmu")
                nc.scalar.activation(
                    nmu[:], msum[:], mybir.ActivationFunctionType.Copy,
                    scale=-1.0 / D,
                )
                sq = lns_pool.tile([128, D], F32, tag="sq")
                vsum = lns_pool.tile([128, 1], F32, tag="vsum")
                nc.scalar.activation(
                    sq[:], x[:], mybir.ActivationFunctionType.Square,
                    bias=nmu[:], accum_out=vsum[:],
                )
                std = lns_pool.tile([128, 1], F32, tag="std")
                nc.scalar.activation(
                    std[:], vsum[:], mybir.ActivationFunctionType.Sqrt,
                    scale=1.0 / D, bias=epst[:],
                )
                rstd = lns_pool.tile([128, 1], F32, tag="rstd")
                nc.vector.reciprocal(rstd[:], std[:])

                xn = lns_pool.tile([128, D], F32, tag="xn")
                nc.vector.tensor_scalar(
                    out=xn[:], in0=x[:], scalar1=nmu[:], scalar2=rstd[:],
                    op0=mybir.AluOpType.add, op1=mybir.AluOpType.mult,
                )
                xg = lns_pool.tile([128, D], F32, tag="xg")
                nc.vector.tensor_mul(xg[:], xn[:], gbt[:])
                xb = lns_pool.tile([128, D], F32, tag="xb")
                nc.vector.tensor_add(xb[:], xg[:], bbt[:])
                nc.sync.dma_start(out[128 * qs : 128 * (qs + 1), :], xb[:])


_NC_CACHE = None


def kernel(q, k, v, Wq, Wk, Wv, Wfc, bfc, gamma, beta):
    global _NC_CACHE
    if _NC_CACHE is None:
        _NC_CACHE = build_kernel()
    nc = _NC_CACHE

    q = np.asarray(q, dtype=np.float32)
    k = np.asarray(k, dtype=np.float32)
    v = np.asarray(v, dtype=np.float32)
    Wq = np.ascontiguousarray(np.asarray(Wq, dtype=np.float32))
    Wk = np.ascontiguousarray(np.asarray(Wk, dtype=np.float32))
    Wv = np.ascontiguousarray(np.asarray(Wv, dtype=np.float32))
    Wfc = np.ascontiguousarray(np.asarray(Wfc, dtype=np.float32))
    bfc = np.asarray(bfc, dtype=np.float32)
    gamma = np.asarray(gamma, dtype=np.float32)
    beta = np.asarray(beta, dtype=np.float32)

    gb = np.ascontiguousarray(np.broadcast_to(gamma, (128, D)))
    bb = np.ascontiguousarray(np.broadcast_to(beta, (128, D)))

    in_maps = []
    for c in range(N_CORES):
        b, r0 = c // 4, (c % 4) * SS
        qs = q[b, r0 : r0 + SS]
        ks = k[b, r0 : r0 + SS]
        vs = v[b, r0 : r0 + SS]
        in_maps.append(
            {
                "qT": np.ascontiguousarray(qs.T),
                "kT": np.ascontiguousarray(ks.T),
                "vT": np.ascontiguousarray(vs.T),
                "qn": np.ascontiguousarray(qs + bfc),
                "Wq": Wq, "Wk": Wk, "Wv": Wv, "Wfc": Wfc,
                "gb": gb, "bb": bb, "onesc": np.ones((128, H), np.float32),
            }
        )

    global _last_in_maps
    _last_in_maps = in_maps
    res = run_bass_kernel_spmd(nc, in_maps, list(range(N_CORES)))
    out = np.empty((B, S, D), dtype=np.float32)
    for c in range(N_CORES):
        b, r0 = c // 4, (c % 4) * SS
        out[b, r0 : r0 + SS] = res.results[c]["out"]
    return out


# revision 28
# speedup vs baseline: 1.0893x; 1.0893x over previous
"""Distributed MultiHeadAttention (+residual, +LayerNorm) Trainium2 kernel.

Problem: B=2, S=2048, D_MODEL=1024, N_HEAD=16, D_K=D_V=64, eps=1e-6.
  qh = q@Wq, kh = k@Wk, vh = v@Wv  (per head)
  attn = softmax(qh·kh^T / 8)
  out = (attn@vh) @ Wfc + bfc + q  -> LayerNorm(gamma, beta)

Sharding: 8 cores; core c owns 512 q-rows of batch c//4 (sequence shard).
Each core projects K/V for its own 512 rows; an AllGather over each
4-core batch group materializes the full-batch K^T/V; attention, fc and
LayerNorm are then fully local (no further collectives).

All matmuls run in float32r (1 cyc/row on PE vs 4 for fp32); the
residual + LayerNorm path stays fp32. Softmax denominators come free
from a ones-column appended to the V stationary tiles.
"""

import sys

sys.path.insert(0, "/opt/trn_rl_repo")

import numpy as np

import concourse.bass as bass
import concourse.tile as tile
from concourse import bacc, mybir
from concourse.bass_utils import run_bass_kernel_spmd

N_CORES = 8
B = 2
S = 2048
D = 1024  # d_model
H = 16  # heads
DK = 64  # head dim
SS = S // 4  # 512 q-rows per core
LN_EPS = 1e-6
F32 = mybir.dt.float32
F32R = mybir.dt.float32r

DEBUG_NO_PBCAST = False

KH_SZ = D * SS  # flat f32 elements of khT_local [1024, 512]
VW = H * 65  # vh row width with per-head ones column baked in
VH_SZ = SS * VW  # flat elements of vh_local [512, 1040]
CC_SZ = KH_SZ + VH_SZ


def build_kernel():
    nc = bacc.Bacc()

    qT = nc.dram_tensor("qT", [D, SS], F32R, kind="ExternalInput")
    kT = nc.dram_tensor("kT", [D, SS], F32R, kind="ExternalInput")
    vT = nc.dram_tensor("vT", [D, SS], F32R, kind="ExternalInput")
    qn = nc.dram_tensor("qn", [SS, D], F32, kind="ExternalInput")  # residual + bfc
    Wq = nc.dram_tensor("Wq", [D, D], F32R, kind="ExternalInput")
    Wk = nc.dram_tensor("Wk", [D, D], F32R, kind="ExternalInput")
    Wv = nc.dram_tensor("Wv", [D, D], F32R, kind="ExternalInput")
    Wfc = nc.dram_tensor("Wfc", [D, D], F32R, kind="ExternalInput")
    gb = nc.dram_tensor("gb", [128, D], F32, kind="ExternalInput")  # gamma bcast
    bb = nc.dram_tensor("bb", [128, D], F32, kind="ExternalInput")  # beta bcast
    onesc = nc.dram_tensor("onesc", [128, H], F32R, kind="ExternalInput")
    out = nc.dram_tensor("out", [SS, D], F32, kind="ExternalOutput")

    cc_in = nc.dram_tensor("cc_in", [CC_SZ], F32R)
    cc_out = nc.dram_tensor("cc_out", [4 * CC_SZ], F32R)

    def cc_in_kh(i):  # [128, 512] view of khT_local rows 128i..128i+127
        return cc_in[128 * SS * i : 128 * SS * (i + 1)].rearrange(
            "(p f) -> p f", f=SS
        )

    def cc_in_vh(s):  # [128, 1040] view of vh_local rows 128s..128s+127
        ofs = KH_SZ + 128 * VW * s
        return cc_in[ofs : ofs + 128 * VW].rearrange("(p f) -> p f", f=VW)

    def cc_out_kh(c, i):  # batch-chunk c, d-rows 128i.. -> [128, 512]
        ofs = c * CC_SZ + 128 * SS * i
        return cc_out[ofs : ofs + 128 * SS].rearrange("(p f) -> p f", f=SS)

    def cc_out_vh(c, s):  # batch-chunk c, k-rows 128s.. -> [128, 1040]
        ofs = c * CC_SZ + KH_SZ + 128 * VW * s
        return cc_out[ofs : ofs + 128 * VW].rearrange("(p f) -> p f", f=VW)

    with tile.TileContext(nc) as tc:
        with (
            tc.tile_pool(name="qhT", bufs=1) as qhT_pool,
            tc.tile_pool(name="outT", bufs=1) as outT_pool,
        ):
            _build_body(nc, tc, qhT_pool, outT_pool, locals())
    nc.compile()
    return nc


def _build_body(nc, tc, qhT_pool, outT_pool, env):
    qT = env["qT"]; kT = env["kT"]; vT = env["vT"]; qn = env["qn"]
    Wq = env["Wq"]; Wk = env["Wk"]; Wv = env["Wv"]; Wfc = env["Wfc"]
    gb = env["gb"]; bb = env["bb"]; out = env["out"]; onesc = env["onesc"]
    cc_in = env["cc_in"]; cc_out = env["cc_out"]
    cc_in_kh = env["cc_in_kh"]; cc_in_vh = env["cc_in_vh"]
    cc_out_kh = env["cc_out_kh"]; cc_out_vh = env["cc_out_vh"]

    if True:
        # ---------------- Phase A: QKV projections of own 512 rows -------
        with (
            tc.tile_pool(name="xin", bufs=1) as xin_pool,
            tc.tile_pool(name="wstat", bufs=6) as wstat_pool,
            tc.tile_pool(name="wmov", bufs=3) as wmov_pool,
            tc.tile_pool(name="stage", bufs=4) as stage_pool,
            tc.tile_pool(name="pp", bufs=1, space="PSUM") as pp_pool,
            tc.tile_pool(name="ppv", bufs=2, space="PSUM") as ppv_pool,
        ):
            # resident input tiles [128, 512] per in-chunk
            qT_t, kT_t, vT_t = [], [], []
            for i in range(8):
                for lst, src, tag in ((qT_t, qT, "q"), (kT_t, kT, "k"), (vT_t, vT, "v")):
                    t = xin_pool.tile([128, SS], F32R, tag=f"x{tag}{i}")
                    nc.sync.dma_start(t[:], src[128 * i : 128 * (i + 1), :])
                    lst.append(t)

            # K projection: khT_local[d, k] = Wk[in, d].T @ kT[in, k]
            # then Q the same; stationary = [128, 128] slices of [128, 512] W tiles
            qhT_tiles = []
            for proj, (W, xt) in enumerate(((Wk, kT_t), (Wq, qT_t))):
                for dblock in range(2):
                    pss = []
                    for dsub in range(4):
                        ps = pp_pool.tile([128, SS], F32, tag=f"pproj{dsub}")
                        pss.append(ps)
                    for i in range(8):
                        wt = wstat_pool.tile([128, 512], F32R, tag="wstat")
                        nc.sync.dma_start(
                            wt[:],
                            W[128 * i : 128 * (i + 1), 512 * dblock : 512 * (dblock + 1)],
                        )
                        for dsub in range(4):
                            nc.tensor.matmul(
                                pss[dsub][:], wt[:, 128 * dsub : 128 * (dsub + 1)],
                                xt[i][:], start=(i == 0), stop=(i == 7),
                            )
                    for dsub in range(4):
                        dchunk = 4 * dblock + dsub
                        if proj == 0:  # K -> stage to DRAM for AllGather
                            st = stage_pool.tile([128, SS], F32R, tag="stagek")
                            nc.vector.tensor_copy(st[:], pss[dsub][:])
                            nc.sync.dma_start(cc_in_kh(dchunk), st[:])
                        else:  # Q -> resident SBUF
                            qt = qhT_pool.tile([128, SS], F32R, tag=f"qh{dchunk}")
                            nc.scalar.copy(qt[:], pss[dsub][:])
                            qhT_tiles.append(qt)

            # V projection: vh_local[k, d] = vT[in, k].T @ Wv[in, d]
            # stationary = vT tile [128 in, 128 k], moving = Wv rows.
            # Staged as [128, 1040] with per-head ones columns baked in so
            # the post-AllGather loads are contiguous.
            for s in range(4):  # k-subtile of own 512 rows
                ps = ppv_pool.tile([128, D], F32, tag="pv")
                for i in range(8):
                    wm = wmov_pool.tile([128, D], F32R, tag="wvmov")
                    nc.sync.dma_start(wm[:], Wv[128 * i : 128 * (i + 1), :])
                    stat = vT_t[i][:, 128 * s : 128 * (s + 1)]
                    nc.tensor.matmul(
                        ps[:, 0:512], stat, wm[:, 0:512], start=(i == 0), stop=(i == 7)
                    )
                    nc.tensor.matmul(
                        ps[:, 512:1024], stat, wm[:, 512:1024], start=(i == 0), stop=(i == 7)
                    )
                st = stage_pool.tile([128, VW], F32R, tag="stagev")
                std = st[:].rearrange("p (h e) -> p h e", h=H)
                nc.vector.tensor_copy(
                    std[:, :, 0:64], ps[:].rearrange("p (h e) -> p h e", h=H)
                )
                nc.sync.dma_start(std[:, :, 64], onesc[:])
                nc.sync.dma_start(cc_in_vh(s), st[:])

        # ---------------- AllGather K^T/V across the batch group ---------
        nc.gpsimd.collective_compute(
            "AllGather",
            mybir.AluOpType.bypass,
            replica_groups=[[0, 1, 2, 3], [4, 5, 6, 7]],
            ins=[cc_in[:]],
            outs=[cc_out[:]],
        )

        # ---------------- Phase B: attention ----------------------------
        # vh_full resident in SBUF with ones column per head: [128, 16*65]
        with (
            tc.tile_pool(name="vh", bufs=1) as vh_pool,
            tc.tile_pool(name="khT", bufs=6) as khT_pool,
            tc.tile_pool(name="pT", bufs=3) as pT_pool,
            tc.tile_pool(name="attn_sc", bufs=3, space="PSUM") as sc_pool,
            tc.tile_pool(name="attn_o", bufs=2, space="PSUM") as po_pool,
            tc.tile_pool(name="dn", bufs=4) as dn_pool,
        ):
            vh_t = []
            for j in range(16):  # global k-tile j = chunk c= j//4, sub s= j%4
                t = vh_pool.tile([128, VW], F32R, tag=f"vh{j}")
                nc.sync.dma_start(t[:], cc_out_vh(j // 4, j % 4))
                vh_t.append(t)

            outT_tiles = []
            for i in range(8):
                oT = outT_pool.tile([128, SS], F32R, tag=f"oT{i}")
                outT_tiles.append(oT)

            for h in range(H):
                # khT stationary tiles for this head-pair, streamed
                if h % 2 == 0:
                    kh_t = []
                    for c in range(4):
                        t = khT_pool.tile([128, SS], F32R, tag="khT")
                        nc.sync.dma_start(t[:], cc_out_kh(c, h // 2))
                        kh_t.append(t)
                hofs = 64 * (h % 2)
                qmov = qhT_tiles[h // 2][hofs : hofs + 64, :]

                po = po_pool.tile([65, SS], F32, tag="po")
                for g in range(8):  # score groups of 2 k-tiles
                    ps = sc_pool.tile([128, 2 * SS], F32, tag="ps")
                    for u in range(2):
                        j = 2 * g + u
                        stat = kh_t[j // 4][hofs : hofs + 64, 128 * (j % 4) : 128 * (j % 4) + 128]
                        nc.tensor.matmul(
                            ps[:, SS * u : SS * (u + 1)], stat, qmov,
                            start=True, stop=True,
                        )
                    pt = pT_pool.tile([128, 2 * SS], F32R, tag="pT")
                    nc.scalar.activation(
                        pt[:], ps[:], mybir.ActivationFunctionType.Exp, scale=0.125
                    )
                    for u in range(2):
                        j = 2 * g + u
                        stat = vh_t[j][:, 65 * h : 65 * h + 65]
                        nc.tensor.matmul(
                            po[:], stat, pt[:, SS * u : SS * (u + 1)],
                            start=(j == 0), stop=(j == 15),
                        )

                # normalize by softmax denominator (row 64 of po)
                rec = dn_pool.tile([1, SS], F32, tag="rec")
                nc.scalar.copy(rec[:], po[64:65, :])
                db = dn_pool.tile([64, SS], F32, tag="db")
                nc.gpsimd.partition_broadcast(db[:], rec[:])
                rb = dn_pool.tile([64, SS], F32, tag="rb")
                nc.vector.reciprocal(rb[:], db[:])
                dst = outT_tiles[h // 2][hofs : hofs + 64, :]
                nc.vector.tensor_mul(dst, po[0:64, :], rb[:])

        # ---------------- Phase C: fc + residual + LayerNorm -------------
        with (
            tc.tile_pool(name="wfc", bufs=8) as wfc_pool,
            tc.tile_pool(name="res", bufs=2) as res_pool,
            tc.tile_pool(name="lnc", bufs=1) as lnc_pool,
            tc.tile_pool(name="lns", bufs=2) as lns_pool,
            tc.tile_pool(name="pfc", bufs=2, space="PSUM") as pfc_pool,
        ):
            gbt = lnc_pool.tile([128, D], F32, tag="gb")
            nc.sync.dma_start(gbt[:], gb[:])
            bbt = lnc_pool.tile([128, D], F32, tag="bb")
            nc.sync.dma_start(bbt[:], bb[:])
            epst = lnc_pool.tile([128, 1], F32, tag="eps")
            nc.vector.memset(epst[:], LN_EPS)

            wfc_t = []  # moving tiles [128 hd-chunk, 1024 m]
            for i in range(8):
                t = wfc_pool.tile([128, D], F32R, tag="wfc")
                nc.sync.dma_start(t[:], Wfc[128 * i : 128 * (i + 1), :])
                wfc_t.append(t)

            for qs in range(4):  # q-subtile of 128 rows
                pf = pfc_pool.tile([128, D], F32, tag="pf")
                for i in range(8):
                    stat = outT_tiles[i][:, 128 * qs : 128 * (qs + 1)]
                    nc.tensor.matmul(
                        pf[:, 0:512], stat, wfc_t[i][:, 0:512],
                        start=(i == 0), stop=(i == 7),
                    )
                    nc.tensor.matmul(
                        pf[:, 512:1024], stat, wfc_t[i][:, 512:1024],
                        start=(i == 0), stop=(i == 7),
                    )

                rt = res_pool.tile([128, D], F32, tag="res")
                nc.sync.dma_start(rt[:], qn[128 * qs : 128 * (qs + 1), :])

                x = lns_pool.tile([128, D], F32, tag="x")
                nc.vector.tensor_add(x[:], pf[:], rt[:])
                msum = lns_pool.tile([128, 1], F32, tag="msum")
                nc.vector.reduce_sum(out=msum[:], in_=x[:], axis=mybir.AxisListType.X)
                nmu = lns_pool.tile([128, 1], F32, tag="nmu")
                nc.scalar.activation(
                    nmu[:], msum[:], mybir.ActivationFunctionType.Copy,
                    scale=-1.0 / D,
                )
                sq = lns_pool.tile([128, D], F32, tag="sq")
                vsum = lns_pool.tile([128, 1], F32, tag="vsum")
                nc.scalar.activation(
                    sq[:], x[:], mybir.ActivationFunctionType.Square,
                    bias=nmu[:], accum_out=vsum[:],
                )
                std = lns_pool.tile([128, 1], F32, tag="std")
                nc.scalar.activation(
                    std[:], vsum[:], mybir.ActivationFunctionType.Sqrt,
                    scale=1.0 / D, bias=epst[:],
                )
                rstd = lns_pool.tile([128, 1], F32, tag="rstd")
                nc.vector.reciprocal(rstd[:], std[:])

                xn = lns_pool.tile([128, D], F32, tag="xn")
                nc.vector.tensor_scalar(
                    out=xn[:], in0=x[:], scalar1=nmu[:], scalar2=rstd[:],
                    op0=mybir.AluOpType.add, op1=mybir.AluOpType.mult,
                )
                xg = lns_pool.tile([128, D], F32, tag="xg")
                nc.vector.tensor_mul(xg[:], xn[:], gbt[:])
                xb = lns_pool.tile([128, D], F32, tag="xb")
                nc.vector.tensor_add(xb[:], xg[:], bbt[:])
                nc.sync.dma_start(out[128 * qs : 128 * (qs + 1), :], xb[:])


_NC_CACHE = None


def kernel(q, k, v, Wq, Wk, Wv, Wfc, bfc, gamma, beta):
    global _NC_CACHE
    if _NC_CACHE is None:
        _NC_CACHE = build_kernel()
    nc = _NC_CACHE

    q = np.asarray(q, dtype=np.float32)
    k = np.asarray(k, dtype=np.float32)
    v = np.asarray(v, dtype=np.float32)
    Wq = np.ascontiguousarray(np.asarray(Wq, dtype=np.float32))
    Wk = np.ascontiguousarray(np.asarray(Wk, dtype=np.float32))
    Wv = np.ascontiguousarray(np.asarray(Wv, dtype=np.float32))
    Wfc = np.ascontiguousarray(np.asarray(Wfc, dtype=np.float32))
    bfc = np.asarray(bfc, dtype=np.float32)
    gamma = np.asarray(gamma, dtype=np.float32)
    beta = np.asarray(beta, dtype=np.float32)

    gb = np.ascontiguousarray(np.broadcast_to(gamma, (128, D)))
    bb = np.ascontiguousarray(np.broadcast_to(beta, (128, D)))

    in_maps = []
    for c in range(N_CORES):
        b, r0 = c // 4, (c % 4) * SS
        qs = q[b, r0 : r0 + SS]
        ks = k[b, r0 : r0 + SS]
        vs = v[b, r0 : r0 + SS]
        in_maps.append(
            {
                "qT": np.ascontiguousarray(qs.T),
                "kT": np.ascontiguousarray(ks.T),
                "vT": np.ascontiguousarray(vs.T),
                "qn": np.ascontiguousarray(qs + bfc),
                "Wq": Wq, "Wk": Wk, "Wv": Wv, "Wfc": Wfc,
                "gb": gb, "bb": bb, "onesc": np.ones((128, H), np.float32),
            }
        )

    global _last_in_maps
    _last_in_maps = in_maps
    res = run_bass_kernel_spmd(nc, in_maps, list(range(N_CORES)))
    out = np.empty((B, S, D), dtype=np.float32)
    for c in range(N_CORES):
        b, r0 = c // 4, (c % 4) * SS
        out[b, r0 : r0 + SS] = res.results[c]["out"]
    return out


# revision 36
# speedup vs baseline: 1.8759x; 1.7221x over previous
"""Distributed MultiHeadAttention (+residual, +LayerNorm) Trainium2 kernel.

Problem: B=2, S=2048, D_MODEL=1024, N_HEAD=16, D_K=D_V=64, eps=1e-6.
  qh = q@Wq, kh = k@Wk, vh = v@Wv  (per head)
  attn = softmax(qh·kh^T / 8)
  out = (attn@vh) @ Wfc + bfc + q  -> LayerNorm(gamma, beta)

Sharding: 8 cores; core c owns 512 q-rows of batch c//4 (sequence shard).
Each core projects K/V for its own 512 rows; an AllGather over each
4-core batch group materializes the full-batch K^T/V; attention, fc and
LayerNorm are then fully local (no further collectives).

All matmul operands are bf16 (fp32 PSUM accumulate); the residual +
LayerNorm path stays fp32. Softmax denominators come free
from a ones-column appended to the V stationary tiles.
"""

import sys

sys.path.insert(0, "/opt/trn_rl_repo")

import ml_dtypes
import numpy as np

import concourse.bass as bass
import concourse.tile as tile
from concourse import bacc, mybir
from concourse.bass_utils import run_bass_kernel_spmd

N_CORES = 8
B = 2
S = 2048
D = 1024  # d_model
H = 16  # heads
DK = 64  # head dim
SS = S // 4  # 512 q-rows per core
LN_EPS = 1e-6
F32 = mybir.dt.float32
BF16 = mybir.dt.bfloat16

DEBUG_NO_PBCAST = False

KH_SZ = D * SS  # flat f32 elements of khT_local [1024, 512]
VW = H * 65  # vh row width with per-head ones column baked in
VH_SZ = SS * VW  # flat elements of vh_local [512, 1040]
CC_SZ = KH_SZ + VH_SZ


def build_kernel():
    nc = bacc.Bacc()

    qT = nc.dram_tensor("qT", [D, SS], BF16, kind="ExternalInput")
    kT = nc.dram_tensor("kT", [D, SS], BF16, kind="ExternalInput")
    vT = nc.dram_tensor("vT", [D, SS], BF16, kind="ExternalInput")
    qn = nc.dram_tensor("qn", [SS, D], F32, kind="ExternalInput")  # residual + bfc
    Wq = nc.dram_tensor("Wq", [D, D], BF16, kind="ExternalInput")
    Wk = nc.dram_tensor("Wk", [D, D], BF16, kind="ExternalInput")
    Wv = nc.dram_tensor("Wv", [D, D], BF16, kind="ExternalInput")
    Wfc = nc.dram_tensor("Wfc", [D, D], BF16, kind="ExternalInput")
    gb = nc.dram_tensor("gb", [128, D], F32, kind="ExternalInput")  # gamma bcast
    bb = nc.dram_tensor("bb", [128, D], F32, kind="ExternalInput")  # beta bcast
    onesc = nc.dram_tensor("onesc", [128, H], BF16, kind="ExternalInput")
    out = nc.dram_tensor("out", [SS, D], F32, kind="ExternalOutput")

    cc_kin = nc.dram_tensor("cc_kin", [KH_SZ], BF16)
    cc_kout = nc.dram_tensor("cc_kout", [4 * KH_SZ], BF16)
    cc_vin = nc.dram_tensor("cc_vin", [VH_SZ], BF16)
    cc_vout = nc.dram_tensor("cc_vout", [4 * VH_SZ], BF16)

    def cc_in_kh(i):  # [128, 512] view of khT_local rows 128i..128i+127
        return cc_kin[128 * SS * i : 128 * SS * (i + 1)].rearrange(
            "(p f) -> p f", f=SS
        )

    def cc_in_vh(s):  # [128, 1040] view of vh_local rows 128s..128s+127
        ofs = 128 * VW * s
        return cc_vin[ofs : ofs + 128 * VW].rearrange("(p f) -> p f", f=VW)

    def cc_out_kh(c, i):  # batch-chunk c, d-rows 128i.. -> [128, 512]
        ofs = c * KH_SZ + 128 * SS * i
        return cc_kout[ofs : ofs + 128 * SS].rearrange("(p f) -> p f", f=SS)

    def cc_out_vh(c, s):  # batch-chunk c, k-rows 128s.. -> [128, 1040]
        ofs = c * VH_SZ + 128 * VW * s
        return cc_vout[ofs : ofs + 128 * VW].rearrange("(p f) -> p f", f=VW)

    with tile.TileContext(nc) as tc:
        with (
            tc.tile_pool(name="qhT", bufs=1) as qhT_pool,
            tc.tile_pool(name="outT", bufs=1) as outT_pool,
        ):
            _build_body(nc, tc, qhT_pool, outT_pool, locals())
    nc.compile()
    return nc


def _build_body(nc, tc, qhT_pool, outT_pool, env):
    qT = env["qT"]; kT = env["kT"]; vT = env["vT"]; qn = env["qn"]
    Wq = env["Wq"]; Wk = env["Wk"]; Wv = env["Wv"]; Wfc = env["Wfc"]
    gb = env["gb"]; bb = env["bb"]; out = env["out"]; onesc = env["onesc"]
    cc_kin = env["cc_kin"]; cc_kout = env["cc_kout"]
    cc_vin = env["cc_vin"]; cc_vout = env["cc_vout"]
    cc_in_kh = env["cc_in_kh"]; cc_in_vh = env["cc_in_vh"]
    cc_out_kh = env["cc_out_kh"]; cc_out_vh = env["cc_out_vh"]

    if True:
        # ---------------- Phase A: QKV projections of own 512 rows -------
        with (
            tc.tile_pool(name="xin", bufs=1) as xin_pool,
            tc.tile_pool(name="wstat", bufs=8) as wstat_pool,
            tc.tile_pool(name="wmov", bufs=8) as wmov_pool,
            tc.tile_pool(name="stage", bufs=4) as stage_pool,
            tc.tile_pool(name="pp", bufs=1, space="PSUM") as pp_pool,
            tc.tile_pool(name="ppv", bufs=2, space="PSUM") as ppv_pool,
        ):
            # resident input tiles [128, 512] per in-chunk
            qT_t, kT_t, vT_t = [], [], []
            for i in range(8):
                for lst, src, tag in ((kT_t, kT, "k"), (vT_t, vT, "v"), (qT_t, qT, "q")):
                    t = xin_pool.tile([128, SS], BF16, tag=f"x{tag}{i}")
                    nc.sync.dma_start(t[:], src[128 * i : 128 * (i + 1), :])
                    lst.append(t)

            # K projection first (feeds AG_kh), V second (AG_vh), Q last.
            qhT_tiles = []
            for proj, (W, xt) in enumerate(((Wk, kT_t),)):
                for dblock in range(2):
                    pss = []
                    for dsub in range(4):
                        ps = pp_pool.tile([128, SS], F32, tag=f"pproj{dsub}")
                        pss.append(ps)
                    for i in range(8):
                        wt = wstat_pool.tile([128, 512], BF16, tag="wstat")
                        nc.sync.dma_start(
                            wt[:],
                            W[128 * i : 128 * (i + 1), 512 * dblock : 512 * (dblock + 1)],
                        )
                        for dsub in range(4):
                            nc.tensor.matmul(
                                pss[dsub][:], wt[:, 128 * dsub : 128 * (dsub + 1)],
                                xt[i][:], start=(i == 0), stop=(i == 7),
                            )
                    for dsub in range(4):
                        dchunk = 4 * dblock + dsub
                        if proj == 0:  # K -> stage to DRAM for AllGather
                            st = stage_pool.tile([128, SS], BF16, tag="stagek")
                            nc.vector.tensor_copy(st[:], pss[dsub][:])
                            nc.sync.dma_start(cc_in_kh(dchunk), st[:])
                        else:  # Q -> resident SBUF
                            qt = qhT_pool.tile([128, SS], BF16, tag=f"qh{dchunk}")
                            nc.scalar.copy(qt[:], pss[dsub][:])
                            qhT_tiles.append(qt)
                if proj == 0:
                    # AllGather K^T as soon as the K staging is in DRAM
                    nc.gpsimd.collective_compute(
                        "AllGather",
                        mybir.AluOpType.bypass,
                        replica_groups=[[0, 1, 2, 3], [4, 5, 6, 7]],
                        ins=[cc_kin[:]],
                        outs=[cc_kout[:]],
                    )

            # V projection: vh_local[k, d] = vT[in, k].T @ Wv[in, d]
            # stationary = vT tile [128 in, 128 k], moving = Wv rows.
            # Staged as [128, 1040] with per-head ones columns baked in so
            # the post-AllGather loads are contiguous.
            for s in range(4):  # k-subtile of own 512 rows
                ps = ppv_pool.tile([128, D], F32, tag="pv")
                for i in range(8):
                    wm = wmov_pool.tile([128, D], BF16, tag="wvmov")
                    nc.sync.dma_start(wm[:], Wv[128 * i : 128 * (i + 1), :])
                    stat = vT_t[i][:, 128 * s : 128 * (s + 1)]
                    nc.tensor.matmul(
                        ps[:, 0:512], stat, wm[:, 0:512], start=(i == 0), stop=(i == 7)
                    )
                    nc.tensor.matmul(
                        ps[:, 512:1024], stat, wm[:, 512:1024], start=(i == 0), stop=(i == 7)
                    )
                st = stage_pool.tile([128, VW], BF16, tag="stagev")
                std = st[:].rearrange("p (h e) -> p h e", h=H)
                nc.vector.tensor_copy(
                    std[:, :, 0:64], ps[:].rearrange("p (h e) -> p h e", h=H)
                )
                nc.sync.dma_start(std[:, :, 64], onesc[:])
                nc.sync.dma_start(cc_in_vh(s), st[:])

            # AllGather V as soon as its staging is in DRAM
            nc.gpsimd.collective_compute(
                "AllGather",
                mybir.AluOpType.bypass,
                replica_groups=[[0, 1, 2, 3], [4, 5, 6, 7]],
                ins=[cc_vin[:]],
                outs=[cc_vout[:]],
            )

            # Q projection last (only needed once scores begin)
            for proj, (W, xt) in enumerate(((Wq, qT_t),)):
                proj = 1
                for dblock in range(2):
                    pss = []
                    for dsub in range(4):
                        ps = pp_pool.tile([128, SS], F32, tag=f"pproj{dsub}")
                        pss.append(ps)
                    for i in range(8):
                        wt = wstat_pool.tile([128, 512], BF16, tag="wstat")
                        nc.sync.dma_start(
                            wt[:],
                            W[128 * i : 128 * (i + 1), 512 * dblock : 512 * (dblock + 1)],
                        )
                        for dsub in range(4):
                            nc.tensor.matmul(
                                pss[dsub][:], wt[:, 128 * dsub : 128 * (dsub + 1)],
                                xt[i][:], start=(i == 0), stop=(i == 7),
                            )
                    for dsub in range(4):
                        dchunk = 4 * dblock + dsub
                        qt = qhT_pool.tile([128, SS], BF16, tag=f"qh{dchunk}")
                        nc.scalar.copy(qt[:], pss[dsub][:])
                        qhT_tiles.append(qt)

        # ---------------- Phase B: attention ----------------------------
        # vh_full resident in SBUF with ones column per head: [128, 16*65]
        with (
            tc.tile_pool(name="vh", bufs=1) as vh_pool,
            tc.tile_pool(name="khT", bufs=8) as khT_pool,
            tc.tile_pool(name="pT", bufs=28) as pT_pool,
            tc.tile_pool(name="attn_sc", bufs=3, space="PSUM") as sc_pool,
            tc.tile_pool(name="attn_o", bufs=1, space="PSUM") as po_pool,
            tc.tile_pool(name="dn", bufs=4) as dn_pool,
        ):
            vh_t = []
            for j in range(16):  # global k-tile j = chunk c= j//4, sub s= j%4
                t = vh_pool.tile([128, VW], BF16, tag=f"vh{j}")
                nc.gpsimd.dma_start(t[:], cc_out_vh(j // 4, j % 4))
                vh_t.append(t)

            outT_tiles = []
            for i in range(8):
                oT = outT_pool.tile([128, SS], BF16, tag=f"oT{i}")
                outT_tiles.append(oT)

            # Heads processed in pairs, interleaved so the PE always has the
            # other head's scores/PV to run while ACT computes an exp.
            for hp in range(H // 2):
                kh_t = []
                for c in range(4):
                    t = khT_pool.tile([128, SS], BF16, tag="khT")
                    nc.sync.dma_start(t[:], cc_out_kh(c, hp))
                    kh_t.append(t)

                pos = []
                for sub in range(2):
                    po = po_pool.tile([65, SS], F32, tag=f"po{sub}")
                    pos.append(po)
                for g in range(8):  # score groups of 2 k-tiles
                    for sub in range(2):
                        h = 2 * hp + sub
                        hofs = 64 * sub
                        qmov = qhT_tiles[hp][hofs : hofs + 64, :]
                        ps = sc_pool.tile([128, 2 * SS], F32, tag="ps")
                        for u in range(2):
                            j = 2 * g + u
                            stat = kh_t[j // 4][hofs : hofs + 64, 128 * (j % 4) : 128 * (j % 4) + 128]
                            nc.tensor.matmul(
                                ps[:, SS * u : SS * (u + 1)], stat, qmov,
                                start=True, stop=True,
                            )
                        pt = pT_pool.tile([128, 2 * SS], BF16, tag="pT")
                        nc.scalar.activation(
                            pt[:], ps[:], mybir.ActivationFunctionType.Exp, scale=0.125
                        )
                        for u in range(2):
                            j = 2 * g + u
                            stat = vh_t[j][:, 65 * h : 65 * h + 65]
                            nc.tensor.matmul(
                                pos[sub][:], stat, pt[:, SS * u : SS * (u + 1)],
                                start=(j == 0), stop=(j == 15),
                            )

                # normalize by softmax denominators (row 64 of po)
                for sub in range(2):
                    h = 2 * hp + sub
                    hofs = 64 * sub
                    rec = dn_pool.tile([1, SS], F32, tag="rec")
                    nc.scalar.copy(rec[:], pos[sub][64:65, :])
                    db = dn_pool.tile([64, SS], F32, tag="db")
                    nc.gpsimd.partition_broadcast(db[:], rec[:])
                    rb = dn_pool.tile([64, SS], F32, tag="rb")
                    nc.vector.reciprocal_approx_fast(rb[:], db[:])
                    dst = outT_tiles[hp][hofs : hofs + 64, :]
                    nc.vector.tensor_mul(dst, pos[sub][0:64, :], rb[:])

        # ---------------- Phase C: fc + residual + LayerNorm -------------
        with (
            tc.tile_pool(name="wfc", bufs=8) as wfc_pool,
            tc.tile_pool(name="res", bufs=2) as res_pool,
            tc.tile_pool(name="lnc", bufs=1) as lnc_pool,
            tc.tile_pool(name="lns", bufs=2) as lns_pool,
            tc.tile_pool(name="pfc", bufs=2, space="PSUM") as pfc_pool,
        ):
            gbt = lnc_pool.tile([128, D], F32, tag="gb")
            nc.sync.dma_start(gbt[:], gb[:])
            bbt = lnc_pool.tile([128, D], F32, tag="bb")
            nc.sync.dma_start(bbt[:], bb[:])
            epst = lnc_pool.tile([128, 1], F32, tag="eps")
            nc.vector.memset(epst[:], LN_EPS)

            wfc_t = []  # moving tiles [128 hd-chunk, 1024 m]
            for i in range(8):
                t = wfc_pool.tile([128, D], BF16, tag="wfc")
                nc.sync.dma_start(t[:], Wfc[128 * i : 128 * (i + 1), :])
                wfc_t.append(t)

            for qs in range(4):  # q-subtile of 128 rows
                pf = pfc_pool.tile([128, D], F32, tag="pf")
                for i in range(8):
                    stat = outT_tiles[i][:, 128 * qs : 128 * (qs + 1)]
                    nc.tensor.matmul(
                        pf[:, 0:512], stat, wfc_t[i][:, 0:512],
                        start=(i == 0), stop=(i == 7),
                    )
                    nc.tensor.matmul(
                        pf[:, 512:1024], stat, wfc_t[i][:, 512:1024],
                        start=(i == 0), stop=(i == 7),
                    )

                rt = res_pool.tile([128, D], F32, tag="res")
                nc.sync.dma_start(rt[:], qn[128 * qs : 128 * (qs + 1), :])

                x = lns_pool.tile([128, D], F32, tag="x")
                nc.vector.tensor_add(x[:], pf[:], rt[:])
                msum = lns_pool.tile([128, 1], F32, tag="msum")
                nc.vector.reduce_sum(out=msum[:], in_=x[:], axis=mybir.AxisListType.X)
                nmu = lns_pool.tile([128, 1], F32, tag="nmu")
                nc.scalar.activation(
                    nmu[:], msum[:], mybir.ActivationFunctionType.Copy,
                    scale=-1.0 / D,
                )
                sq = lns_pool.tile([128, D], F32, tag="sq")
                vsum = lns_pool.tile([128, 1], F32, tag="vsum")
                nc.scalar.activation(
                    sq[:], x[:], mybir.ActivationFunctionType.Square,
                    bias=nmu[:], accum_out=vsum[:],
                )
                std = lns_pool.tile([128, 1], F32, tag="std")
                nc.scalar.activation(
                    std[:], vsum[:], mybir.ActivationFunctionType.Sqrt,
                    scale=1.0 / D, bias=epst[:],
                )
                rstd = lns_pool.tile([128, 1], F32, tag="rstd")
                nc.vector.reciprocal(rstd[:], std[:])

                xn = lns_pool.tile([128, D], F32, tag="xn")
                nc.vector.tensor_scalar(
                    out=xn[:], in0=x[:], scalar1=nmu[:], scalar2=rstd[:],
                    op0=mybir.AluOpType.add, op1=mybir.AluOpType.mult,
                )
                xg = lns_pool.tile([128, D], F32, tag="xg")
                nc.vector.tensor_mul(xg[:], xn[:], gbt[:])
                xb = lns_pool.tile([128, D], F32, tag="xb")
                nc.vector.tensor_add(xb[:], xg[:], bbt[:])
                nc.sync.dma_start(out[128 * qs : 128 * (qs + 1), :], xb[:])


_NC_CACHE = None


def kernel(q, k, v, Wq, Wk, Wv, Wfc, bfc, gamma, beta):
    global _NC_CACHE
    if _NC_CACHE is None:
        _NC_CACHE = build_kernel()
    nc = _NC_CACHE

    bf16 = ml_dtypes.bfloat16
    q = np.asarray(q, dtype=np.float32)
    k = np.asarray(k, dtype=np.float32)
    v = np.asarray(v, dtype=np.float32)
    Wq = np.ascontiguousarray(np.asarray(Wq, dtype=np.float32).astype(bf16))
    Wk = np.ascontiguousarray(np.asarray(Wk, dtype=np.float32).astype(bf16))
    Wv = np.ascontiguousarray(np.asarray(Wv, dtype=np.float32).astype(bf16))
    Wfc = np.ascontiguousarray(np.asarray(Wfc, dtype=np.float32).astype(bf16))
    bfc = np.asarray(bfc, dtype=np.float32)
    gamma = np.asarray(gamma, dtype=np.float32)
    beta = np.asarray(beta, dtype=np.float32)

    gb = np.ascontiguousarray(np.broadcast_to(gamma, (128, D)))
    bb = np.ascontiguousarray(np.broadcast_to(beta, (128, D)))

    in_maps = []
    for c in range(N_CORES):
        b, r0 = c // 4, (c % 4) * SS
        qs = q[b, r0 : r0 + SS]
        ks = k[b, r0 : r0 + SS]
        vs = v[b, r0 : r0 + SS]
        in_maps.append(
            {
                "qT": np.ascontiguousarray(qs.T.astype(bf16)),
                "kT": np.ascontiguousarray(ks.T.astype(bf16)),
                "vT": np.ascontiguousarray(vs.T.astype(bf16)),
                "qn": np.ascontiguousarray(qs + bfc),
                "Wq": Wq, "Wk": Wk, "Wv": Wv, "Wfc": Wfc,
                "gb": gb, "bb": bb, "onesc": np.ones((128, H), bf16),
            }
        )

    global _last_in_maps
    _last_in_maps = in_maps
    res = run_bass_kernel_spmd(nc, in_maps, list(range(N_CORES)))
    out = np.empty((B, S, D), dtype=np.float32)
    for c in range(N_CORES):
        b, r0 = c // 4, (c % 4) * SS
        out[b, r0 : r0 + SS] = res.results[c]["out"]
    return out
